# revision 16
# baseline (speedup 1.0000x reference)
"""Trainium2 Bass kernel for nn_ClusterLoss_Regr (topk_masking).

Computes  mean_b(128 - max_p((128 - d[b,p]) * [|proto[p] - label[b]| <= 0.5]))
for d: [8192, 4096] f32, labels: [8192] f32, proto: [4096] f32 -> scalar f32.

Sharding: data-parallel over the batch axis across 8 NeuronCores (1024 rows
per core); proto_classes replicated; final mean on host.

Device strategy (memory-bound):
  - d is staged to HBM as bf16 (halves HBM traffic; rel tolerance 2e-2 vs
    bf16's 2^-9 rounding).  Since f32/bf16 rounding is monotone,
    max_p f32(128-d_p) = f32(128 - min_p d_p), so the device computes the
    masked row MIN of d directly and the host reconstructs
    f32(128 - f32(128 - dmin)) bit-exactly mirroring the reference chain.
  - One fused custom-DVE op per row-tile:
        v[p,k]   = select(0.5 >= |proto[k] - label[p]|, d[p,k], BIG)
        accum[p] = min_k v[p,k]
    The op is registered with a hand-written 2X_1PORT uop program (packed
    bf16 pairs, SRC_0_HI/SRC_1_HI crossbar inputs, dual lo/hi ALU chains in
    8 stages) so the DVE runs at 2 elem/lane/cycle; the label rides latched
    swap-flops at the two ABSOLUTE_DIFF stages.
  - proto is staged pre-replicated [128, 4096] bf16 by the host (1 MB extra
    HBM read, fully overlapped) replacing the baseline's 14.5 us GPSIMD
    partition_broadcast prologue.
  - raw Bass with manual semaphores: head DMAs ride the scalar HWDGE ring;
    d-tile DMAs stream back-to-back on the sync ring; DVE op t is gated only
    on d-tile t; the last row-tile is processed as 4 quarter-width ops to
    minimise the tail.
Host: gather [8192] row minima, loss = mean(128 - (128 - dmin)) accumulated
in f64, cast to f32.
"""

import numpy as np

B, P = 8192, 4096
NCORES = 8
BSH = B // NCORES  # 1024 rows per core
RT = BSH // 128    # 8 row-tiles of 128 rows
MAX_DIST = np.float32(128.0)
BIG = 2.0          # "unmasked" fill; any value > max(d)=1.0 works
USE_2X = True

_cache: dict = {}


def _ensure_path():
    try:
        import concourse.bass  # noqa: F401
    except ImportError:
        import sys

        for p in ("/opt/trn_rl_repo",):
            if p not in sys.path:
                sys.path.insert(0, p)


def _build_2x_uops():
    """Hand-written 2X_1PORT program: 2 states (seed, steady) rate-matched
    to lower()'s 1x program, shaped after the stock tensor_mask 2x program
    (slot 105 of the gen3 firmware table): SRC_0 rides input 0 (read at b0
    as PREV_ALU_OUT, captured into lane 0), both write halves enabled.

    Lanes: L0=SRC_1 (b0 only; then captures Src0, then v_lo), L1=CONST_0
    (label), L2=CONST_1 (0.5), L3=CONST_2 (BIG), L4=SRC_0_HI (then v_hi
    from b6), L5=SRC_1_HI.

      b0: ad_lo = |Src1 - label|   [d0 <- Src0]
      b1: c_lo  = 0.5 >= ad_lo
      b2: v_lo  = sel(c_lo, Src0, BIG)
      b3: ad_hi = |Src1_HI - label|  [d0 <- v_lo]
      b4: c_hi  = 0.5 >= ad_hi
      b5: v_hi  = sel(c_hi, Src0_HI, BIG)
      b6: w     = min(v_lo, v_hi)
      b7: acc   = min(acc, w)  -> written to both output halves; the last
                  written word of the stream is the masked row min.  (The
                  persistent-accumulator readback is dead in 2X mode, so the
                  running min is streamed through the write port instead.)
    """
    from concourse.dve_uop import (
        AluInp,
        AluOp,
        DelayInp,
        InpSel,
        OutPath,
        OutSel,
        Trigger,
        UopConfig,
        UopDpConfig,
    )

    ENABLE = 1
    P_AD = AluInp.PREV_ALU_OUT
    CUR = AluInp.CURR_ALU_OUT
    D = [
        AluInp.PREV_DELAY_0,
        AluInp.PREV_DELAY_1,
        AluInp.PREV_DELAY_2,
        AluInp.PREV_DELAY_3,
        AluInp.PREV_DELAY_4,
        AluInp.PREV_DELAY_5,
    ]
    SRC_DONE = (Trigger.SRC_TENSOR_DONE, Trigger.NONE, Trigger.NONE)
    COUNT_ONCE = (Trigger.COUNT, Trigger.NONE, Trigger.NONE)

    def wire_inputs(u):
        u.enable_input(InpSel.SRC_0, 0)      # input 0 -> b0's PREV_ALU_OUT
        u.enable_input(InpSel.SRC_1, 1)      # lane 0
        u.enable_input(InpSel.CONST_0, 2)    # lane 1: label
        u.enable_input(InpSel.CONST_1, 3)    # lane 2: 0.5
        u.enable_input(InpSel.CONST_2, 4)    # lane 3: BIG
        u.enable_input(InpSel.SRC_0_HI, 5)   # lane 4
        u.enable_input(InpSel.SRC_1_HI, 6)   # lane 5

    def steady_blocks():
        dp = [UopDpConfig() for _ in range(8)]
        for i in range(8):
            dp[i].pass_through_delay(1, 2, 3, 5)
            if i not in (0, 3):
                dp[i].pass_through_delay(0)
            if i != 6:
                dp[i].pass_through_delay(4)
        dp[0].enable_alu(AluOp.ABSOLUTE_DIFF, D[0], D[1])
        dp[0].enable_delay_from_src(DelayInp.PREV_ALU_OUT, 0)  # Src0
        dp[1].enable_alu(AluOp.IS_GE, D[2], P_AD)
        dp[2].enable_alu(AluOp.SELECT, D[3], D[0])  # falsy->BIG, truthy->Src0
        dp[3].enable_alu(AluOp.ABSOLUTE_DIFF, D[5], D[1])
        dp[3].enable_delay_from_src(DelayInp.PREV_ALU_OUT, 0)  # v_lo
        dp[4].enable_alu(AluOp.IS_GE, D[2], P_AD)
        dp[5].enable_alu(AluOp.SELECT, D[3], D[4])
        dp[6].enable_alu(AluOp.MIN, D[0], P_AD)
        dp[7].enable_alu(AluOp.MIN, CUR, P_AD)
        return dp

    # --- state 0: seed — scan state (b7's out-flop) <- BIG (CONST_2, lane 3).
    sd = UopConfig(trigger=COUNT_ONCE, repeat_count=1, next_uop=(1, 0, 0))
    wire_inputs(sd)
    dp = steady_blocks()
    dp[7] = UopDpConfig()
    dp[7].pass_through_delay(0, 1, 2, 3, 4, 5)
    dp[7].enable_alu(AluOp.BYPASS, D[3], D[3])
    sd.datapath_config = dp

    # --- state 1: steady.  Both write halves carry the running min.
    st = UopConfig(
        trigger=SRC_DONE,
        require_inp0=ENABLE,
        require_inp1=ENABLE,
    )
    wire_inputs(st)
    st.datapath_config = steady_blocks()
    st.enable_output(OutSel.ALU_OUT, OutPath.WR0_LO)
    st.enable_output(OutSel.ALU_OUT, OutPath.WR0_HI)
    return [sd, st]


def _register_dve_op():
    """Register the fused |proto-label|-mask + min-reduce op, with a
    hand-authored 2X_1PORT perf-mode program. Idempotent."""
    from concourse import dve_ops
    from concourse.dve_spec import (
        C0,
        C1,
        C2,
        AluOp,
        Bin,
        Spec,
        Src0,
        Src1,
        lower,
        scan,
        select,
    )
    from concourse.dve_uop import DveOpSpec

    name = "CLUSTER_MASKMIN_ANT"
    for op in dve_ops.OPS:
        if op.name == name:
            return op

    def _ref(in0, in1, s0, s1, imm2):
        mask = np.abs(in1.astype(np.float32) - np.asarray(s0, np.float32)) <= (
            np.float32(s1)
        )
        o = np.where(mask, in0.astype(np.float32), np.float32(imm2)).astype(
            np.float32
        )
        return np.minimum.accumulate(o, axis=-1)

    # body: running min of select(0.5 >= |Src1 - label|, Src0, BIG); the
    # last element of the output stream is the masked row min.
    spec = Spec(
        body=scan(
            AluOp.MIN,
            select(C1 >= Bin(AluOp.ABSOLUTE_DIFF, Src1, C0), Src0, C2),
            init=C2,
        ),
        reference=_ref,
    )

    class _DveOp2x(dve_ops.DveOp):
        def compile(self, ver):
            key = (self.name, ver)
            if (r := dve_ops._COMPILE_CACHE.get(key)) is not None:
                return r
            uops = lower(self.spec, ver=ver)
            uops_2x = None
            if USE_2X and ver == "v3":
                uops_2x = _build_2x_uops()
                assert len(uops_2x) == len(uops), (len(uops_2x), len(uops))
            result = DveOpSpec(
                name=self.name,
                opcode=dve_ops.get_dve_sub_opcode(self.name),
                uops=uops,
                uops_2x=uops_2x,
                perf_max=1 if uops_2x is not None else 0,
                rd1_en=True,
            )
            dve_ops._COMPILE_CACHE[key] = result
            return result

    shas: dict = {}
    op = _DveOp2x(name, spec, subdim=False, uops_sha=shas)
    dve_ops.OPS.append(op)
    row = dve_ops._CUSTOM_DVE_ROW_BASE + len(dve_ops.OPS) - 1
    dve_ops._SUB_OPCODE_FOR_NAME[name] = row
    dve_ops.CUSTOM_DVE_SPECS[name] = spec
    for ver in ("v3", "v4"):
        shas[ver] = op.compile(ver).sha(ver) if ver == "v3" else ""
    return op


def _get_bass():
    if "nc" in _cache:
        return _cache["nc"]
    _ensure_path()
    import concourse.bacc as bacc
    import concourse.mybir as mybir

    op = _register_dve_op()
    f32 = mybir.dt.float32
    bf16 = mybir.dt.bfloat16
    nc = bacc.Bacc(
        "TRN2", target_bir_lowering=False, debug=False, num_devices=NCORES
    )
    d_ap = nc.dram_tensor("d", [128, RT * P], bf16, kind="ExternalInput").ap()
    lab_ap = nc.dram_tensor("labels_col", [128, RT], f32, kind="ExternalInput").ap()
    pb_ap = nc.dram_tensor("proto_bc", [128, P], bf16, kind="ExternalInput").ap()
    # The LAST row-tile is processed as NSPLIT quarter-width ops so the
    # final DVE op rides only a quarter tile behind the last DMA byte.
    NSPLIT = 4
    ND = RT - 1 + NSPLIT      # number of d DMAs == number of DVE ops
    NV = ND
    # rowmin[:, 2i:2i+2] <- the last written word of op i's scan stream;
    # column 2i+1 is the final running-min in both 1x and 2x modes.
    out_ap = nc.dram_tensor("rowmin", [128, 2 * ND], bf16, kind="ExternalOutput").ap()

    proto_tile = nc.alloc_sbuf_tensor("proto_tile", [128, P], bf16).ap()
    labels_tile = nc.alloc_sbuf_tensor("labels_tile", [128, RT], f32).ap()
    # per-tile scan output (ops into the same tile use disjoint col ranges)
    scr = [nc.alloc_sbuf_tensor(f"scr{t}", [128, P], bf16).ap() for t in range(RT)]
    dbig = nc.alloc_sbuf_tensor("dbig", [128, RT * P], bf16).ap()
    # gpsimd bf16 partition-broadcast probe: duration read from the profile;
    # nothing depends on it.
    prow = nc.alloc_sbuf_tensor("prow", [1, P], bf16).ap()
    probe = nc.alloc_sbuf_tensor("probe", [128, P], bf16).ap()

    H = P // NSPLIT           # split width of the last tile

    # (tile, col_lo, width) in stream order; col offsets are into dbig
    work = [(t, 0, P) for t in range(RT - 1)]
    for s in range(NSPLIT):
        work.append((RT - 1, s * H, H))
    # d DMAs: (dbig_col_lo, width, first_op_gated) — tiles 2+3 and 4+5 fused
    dma_plan = [
        (0 * P, P, 0),
        (1 * P, P, 1),
        (2 * P, 2 * P, 2),
        (4 * P, 2 * P, 4),
        (6 * P, P, 6),
    ] + [((RT - 1) * P + s * H, H, RT - 1 + s) for s in range(NSPLIT)]
    # op i is gated on the dma covering it
    dma_of_op = {}
    for di, (lo, w, first_op) in enumerate(dma_plan):
        for oi in range(len(work)):
            t, olo, ow = work[oi]
            a = t * P + olo
            if lo <= a < lo + w:
                dma_of_op[oi] = max(dma_of_op.get(oi, 0), di)

    # One semaphore per DMA (a shared sem with cumulative thresholds can
    # fire early when the 16 SDMA engines skew; a dedicated sem == 16 is
    # exact).
    d_sems = [nc.alloc_semaphore(f"d_sem{i}") for i in range(len(dma_plan))]
    pb_sem = nc.alloc_semaphore("pb_sem")
    prow_sem = nc.alloc_semaphore("prow_sem")
    probe_sem = nc.alloc_semaphore("probe_sem")
    lab_sem = nc.alloc_semaphore("lab_sem")
    out_sem = nc.alloc_semaphore("out_sem")
    dve_sem = nc.alloc_semaphore("dve_sem")

    with nc.Block() as block:

        @block.sync
        def _(sync):
            # Head DMAs lead the single HWDGE stream: a separate scalar-ring
            # DMA is starved ~1:12 behind the d-stream by the per-queue-row
            # round-robin (measured: 1 MB proto took 17 us to land).
            sync.dma_start(proto_tile[:], pb_ap[:]).then_inc(pb_sem, 16)
            sync.dma_start(labels_tile[:], lab_ap[:]).then_inc(lab_sem, 16)
            sync.dma_start(prow[:], pb_ap[0:1, :]).then_inc(prow_sem, 16)
            for i, (lo, w, _fo) in enumerate(dma_plan):
                sync.dma_start(
                    dbig[:, lo : lo + w], d_ap[:, lo : lo + w]
                ).then_inc(d_sems[i], 16)
            # Gather each op's final scan word as soon as that op retires;
            # only the last one's completion latency lands in the tail.
            for i, (t, lo, w) in enumerate(work):
                sync.wait_ge(dve_sem, i + 1)
                sync.dma_start(
                    out_ap[:, 2 * i : 2 * i + 2],
                    scr[t][:, lo + w - 2 : lo + w],
                ).then_inc(out_sem, 16)
            sync.wait_ge(out_sem, 16 * ND)
            sync.wait_ge(probe_sem, 1)
            # Reset all kernel semaphores so re-executing the loaded NEFF
            # behaves identically to the first run.
            all_sems = sorted(
                s.num
                for s in [
                    *d_sems, pb_sem, prow_sem, probe_sem, lab_sem, out_sem,
                    dve_sem,
                ]
            )
            lo = prev = all_sems[0]
            for n in all_sems[1:] + [None]:
                if n is not None and n == prev + 1:
                    prev = n
                    continue
                sync.sem_clear(range(lo, prev + 1))
                if n is not None:
                    lo = prev = n

        @block.gpsimd
        def _(gpsimd):
            gpsimd.wait_ge(prow_sem, 16)
            gpsimd.partition_broadcast(probe[:], prow[:]).then_inc(probe_sem, 1)

        @block.vector
        def _(vector):
            vector.wait_ge(pb_sem, 16)
            vector.wait_ge(lab_sem, 16)
            for i, (t, lo, w) in enumerate(work):
                vector.wait_ge(d_sems[dma_of_op[i]], 16)
                inst = nc.vector._custom_dve(
                    op,
                    out=scr[t][:, lo : lo + w],
                    in0=dbig[:, t * P + lo : t * P + lo + w],
                    in1=proto_tile[:, lo : lo + w],
                    s0=labels_tile[:, t : t + 1],
                    s1=0.5,
                    imm2=float(BIG),
                )
                if USE_2X:
                    inst.ins.perf_max = 1
                inst.then_inc(dve_sem, 1)

    nc.compile()
    _cache["nc"] = nc
    return nc


def _prep_inputs(min_distances, labels, proto_classes):
    import ml_dtypes

    bf16 = ml_dtypes.bfloat16
    d = np.asarray(min_distances, dtype=np.float32).astype(bf16)
    proto = np.asarray(proto_classes, dtype=np.float32).astype(bf16)
    proto_bc = np.ascontiguousarray(np.broadcast_to(proto[None, :], (128, P)))
    labf = np.asarray(labels, dtype=np.float32)
    in_maps = []
    for c in range(NCORES):
        dsh = np.ascontiguousarray(
            d[c * BSH : (c + 1) * BSH]
            .reshape(RT, 128, P)
            .transpose(1, 0, 2)
            .reshape(128, RT * P)
        )
        lsh = np.ascontiguousarray(
            labf[c * BSH : (c + 1) * BSH].reshape(RT, 128).T
        )
        in_maps.append({"d": dsh, "labels_col": lsh, "proto_bc": proto_bc})
    return in_maps


def _run_device(min_distances, labels, proto_classes, trace=False):
    nc = _get_bass()
    from concourse.bass_utils import run_bass_kernel_spmd

    in_maps = _prep_inputs(min_distances, labels, proto_classes)
    return run_bass_kernel_spmd(
        nc, in_maps, core_ids=list(range(NCORES)), trace=trace
    )


def kernel(min_distances, labels, proto_classes):
    res = _run_device(min_distances, labels, proto_classes).results
    # rowmin[:, 2i+1] = final scan value of op i.  Ops 0..RT-2 are tiles
    # 0..RT-2; ops RT-1.. are quarters of tile RT-1 (combine by min).
    # Row = 1024*c + 128*t + p.  bf16 is exact here: a min of bf16 values.
    stats = np.stack(
        [
            np.asarray(res[c]["rowmin"])[:, 1::2].astype(np.float32)
            for c in range(NCORES)
        ]
    )
    t_last = stats[:, :, RT - 1 :].min(axis=2)
    rowmin = np.concatenate([stats[:, :, : RT - 1], t_last[:, :, None]], axis=2)
    rowmin = rowmin.transpose(0, 2, 1).reshape(B).astype(np.float32)
    # mirror the reference's f32 rounding chain exactly:
    # loss_row = f32(128 - f32(128 - dmin))
    inv = (MAX_DIST - rowmin).astype(np.float32)
    loss_rows = (MAX_DIST - inv).astype(np.float32)
    return np.array(loss_rows.mean(dtype=np.float64), dtype=np.float32)


# revision 19
# speedup vs baseline: 1.0558x; 1.0558x over previous
"""Trainium2 Bass kernel for nn_ClusterLoss_Regr (topk_masking).

Computes  mean_b(128 - max_p((128 - d[b,p]) * [|proto[p] - label[b]| <= 0.5]))
for d: [8192, 4096] f32, labels: [8192] f32, proto: [4096] f32 -> scalar f32.

Sharding: data-parallel over the batch axis across 8 NeuronCores (1024 rows
per core); proto_classes replicated; final mean on host.

Device strategy (memory-bound):
  - d is staged to HBM as bf16 (halves HBM traffic; rel tolerance 2e-2 vs
    bf16's 2^-9 rounding).  Since f32/bf16 rounding is monotone,
    max_p f32(128-d_p) = f32(128 - min_p d_p), so the device computes the
    masked row MIN of d directly and the host reconstructs
    f32(128 - f32(128 - dmin)) bit-exactly mirroring the reference chain.
  - One fused custom-DVE op per row-tile:
        v[p,k]   = select(0.5 >= |proto[k] - label[p]|, d[p,k], BIG)
        accum[p] = min_k v[p,k]
    The op is registered with a hand-written 2X_1PORT uop program (packed
    bf16 pairs, SRC_0_HI/SRC_1_HI crossbar inputs, dual lo/hi ALU chains in
    8 stages) so the DVE runs at 2 elem/lane/cycle; the label rides latched
    swap-flops at the two ABSOLUTE_DIFF stages.
  - proto is staged pre-replicated [128, 4096] bf16 by the host (1 MB extra
    HBM read, fully overlapped) replacing the baseline's 14.5 us GPSIMD
    partition_broadcast prologue.
  - raw Bass with manual semaphores: head DMAs ride the scalar HWDGE ring;
    d-tile DMAs stream back-to-back on the sync ring; DVE op t is gated only
    on d-tile t; the last row-tile is processed as 4 quarter-width ops to
    minimise the tail.
Host: gather [8192] row minima, loss = mean(128 - (128 - dmin)) accumulated
in f64, cast to f32.
"""

import numpy as np

B, P = 8192, 4096
NCORES = 8
BSH = B // NCORES  # 1024 rows per core
RT = BSH // 128    # 8 row-tiles of 128 rows
MAX_DIST = np.float32(128.0)
BIG = 2.0          # "unmasked" fill; any value > max(d)=1.0 works
USE_2X = True

_cache: dict = {}


def _ensure_path():
    try:
        import concourse.bass  # noqa: F401
    except ImportError:
        import sys

        for p in ("/opt/trn_rl_repo",):
            if p not in sys.path:
                sys.path.insert(0, p)


def _build_2x_uops():
    """Hand-written 2X_1PORT program: 2 states (seed, steady) rate-matched
    to lower()'s 1x program, shaped after the stock tensor_mask 2x program
    (slot 105 of the gen3 firmware table): SRC_0 rides input 0 (read at b0
    as PREV_ALU_OUT, captured into lane 0), both write halves enabled.

    Lanes: L0=SRC_1 (b0 only; then captures Src0, then v_lo), L1=CONST_0
    (label), L2=CONST_1 (0.5), L3=CONST_2 (BIG), L4=SRC_0_HI (then v_hi
    from b6), L5=SRC_1_HI.

      b0: ad_lo = |Src1 - label|   [d0 <- Src0]
      b1: c_lo  = 0.5 >= ad_lo
      b2: v_lo  = sel(c_lo, Src0, BIG)
      b3: ad_hi = |Src1_HI - label|  [d0 <- v_lo]
      b4: c_hi  = 0.5 >= ad_hi
      b5: v_hi  = sel(c_hi, Src0_HI, BIG)
      b6: w     = min(v_lo, v_hi)
      b7: acc   = min(acc, w)  -> written to both output halves; the last
                  written word of the stream is the masked row min.  (The
                  persistent-accumulator readback is dead in 2X mode, so the
                  running min is streamed through the write port instead.)
    """
    from concourse.dve_uop import (
        AluInp,
        AluOp,
        DelayInp,
        InpSel,
        OutPath,
        OutSel,
        Trigger,
        UopConfig,
        UopDpConfig,
    )

    ENABLE = 1
    P_AD = AluInp.PREV_ALU_OUT
    CUR = AluInp.CURR_ALU_OUT
    D = [
        AluInp.PREV_DELAY_0,
        AluInp.PREV_DELAY_1,
        AluInp.PREV_DELAY_2,
        AluInp.PREV_DELAY_3,
        AluInp.PREV_DELAY_4,
        AluInp.PREV_DELAY_5,
    ]
    SRC_DONE = (Trigger.SRC_TENSOR_DONE, Trigger.NONE, Trigger.NONE)
    COUNT_ONCE = (Trigger.COUNT, Trigger.NONE, Trigger.NONE)

    def wire_inputs(u):
        u.enable_input(InpSel.SRC_0, 0)      # input 0 -> b0's PREV_ALU_OUT
        u.enable_input(InpSel.SRC_1, 1)      # lane 0
        u.enable_input(InpSel.CONST_0, 2)    # lane 1: label
        u.enable_input(InpSel.CONST_1, 3)    # lane 2: 0.5
        u.enable_input(InpSel.CONST_2, 4)    # lane 3: BIG
        u.enable_input(InpSel.SRC_0_HI, 5)   # lane 4
        u.enable_input(InpSel.SRC_1_HI, 6)   # lane 5

    def steady_blocks():
        dp = [UopDpConfig() for _ in range(8)]
        for i in range(8):
            dp[i].pass_through_delay(1, 2, 3, 5)
            if i not in (0, 3):
                dp[i].pass_through_delay(0)
            if i != 6:
                dp[i].pass_through_delay(4)
        dp[0].enable_alu(AluOp.ABSOLUTE_DIFF, D[0], D[1])
        dp[0].enable_delay_from_src(DelayInp.PREV_ALU_OUT, 0)  # Src0
        dp[1].enable_alu(AluOp.IS_GE, D[2], P_AD)
        dp[2].enable_alu(AluOp.SELECT, D[3], D[0])  # falsy->BIG, truthy->Src0
        dp[3].enable_alu(AluOp.ABSOLUTE_DIFF, D[5], D[1])
        dp[3].enable_delay_from_src(DelayInp.PREV_ALU_OUT, 0)  # v_lo
        dp[4].enable_alu(AluOp.IS_GE, D[2], P_AD)
        dp[5].enable_alu(AluOp.SELECT, D[3], D[4])
        dp[6].enable_alu(AluOp.MIN, D[0], P_AD)
        dp[7].enable_alu(AluOp.MIN, CUR, P_AD)
        return dp

    # --- state 0: seed — scan state (b7's out-flop) <- BIG (CONST_2, lane 3).
    sd = UopConfig(trigger=COUNT_ONCE, repeat_count=1, next_uop=(1, 0, 0))
    wire_inputs(sd)
    dp = steady_blocks()
    dp[7] = UopDpConfig()
    dp[7].pass_through_delay(0, 1, 2, 3, 4, 5)
    dp[7].enable_alu(AluOp.BYPASS, D[3], D[3])
    sd.datapath_config = dp

    # --- state 1: steady.  Both write halves carry the running min.
    st = UopConfig(
        trigger=SRC_DONE,
        require_inp0=ENABLE,
        require_inp1=ENABLE,
    )
    wire_inputs(st)
    st.datapath_config = steady_blocks()
    st.enable_output(OutSel.ALU_OUT, OutPath.WR0_LO)
    st.enable_output(OutSel.ALU_OUT, OutPath.WR0_HI)
    return [sd, st]


def _register_dve_op():
    """Register the fused |proto-label|-mask + min-reduce op, with a
    hand-authored 2X_1PORT perf-mode program. Idempotent."""
    from concourse import dve_ops
    from concourse.dve_spec import (
        C0,
        C1,
        C2,
        AluOp,
        Bin,
        Spec,
        Src0,
        Src1,
        lower,
        scan,
        select,
    )
    from concourse.dve_uop import DveOpSpec

    name = "CLUSTER_MASKMIN_ANT"
    for op in dve_ops.OPS:
        if op.name == name:
            return op

    def _ref(in0, in1, s0, s1, imm2):
        mask = np.abs(in1.astype(np.float32) - np.asarray(s0, np.float32)) <= (
            np.float32(s1)
        )
        o = np.where(mask, in0.astype(np.float32), np.float32(imm2)).astype(
            np.float32
        )
        return np.minimum.accumulate(o, axis=-1)

    # body: running min of select(0.5 >= |Src1 - label|, Src0, BIG); the
    # last element of the output stream is the masked row min.
    spec = Spec(
        body=scan(
            AluOp.MIN,
            select(C1 >= Bin(AluOp.ABSOLUTE_DIFF, Src1, C0), Src0, C2),
            init=C2,
        ),
        reference=_ref,
    )

    class _DveOp2x(dve_ops.DveOp):
        def compile(self, ver):
            key = (self.name, ver)
            if (r := dve_ops._COMPILE_CACHE.get(key)) is not None:
                return r
            uops = lower(self.spec, ver=ver)
            uops_2x = None
            if USE_2X and ver == "v3":
                uops_2x = _build_2x_uops()
                assert len(uops_2x) == len(uops), (len(uops_2x), len(uops))
            result = DveOpSpec(
                name=self.name,
                opcode=dve_ops.get_dve_sub_opcode(self.name),
                uops=uops,
                uops_2x=uops_2x,
                perf_max=1 if uops_2x is not None else 0,
                rd1_en=True,
            )
            dve_ops._COMPILE_CACHE[key] = result
            return result

    shas: dict = {}
    op = _DveOp2x(name, spec, subdim=False, uops_sha=shas)
    dve_ops.OPS.append(op)
    row = dve_ops._CUSTOM_DVE_ROW_BASE + len(dve_ops.OPS) - 1
    dve_ops._SUB_OPCODE_FOR_NAME[name] = row
    dve_ops.CUSTOM_DVE_SPECS[name] = spec
    for ver in ("v3", "v4"):
        shas[ver] = op.compile(ver).sha(ver) if ver == "v3" else ""
    return op


def _get_bass():
    if "nc" in _cache:
        return _cache["nc"]
    _ensure_path()
    import concourse.bacc as bacc
    import concourse.mybir as mybir

    op = _register_dve_op()
    f32 = mybir.dt.float32
    bf16 = mybir.dt.bfloat16
    nc = bacc.Bacc(
        "TRN2", target_bir_lowering=False, debug=False, num_devices=NCORES
    )
    d_ap = nc.dram_tensor("d", [128, RT * P], bf16, kind="ExternalInput").ap()
    lab_ap = nc.dram_tensor("labels_col", [128, RT], f32, kind="ExternalInput").ap()
    pb_ap = nc.dram_tensor("proto_bc", [128, P], bf16, kind="ExternalInput").ap()
    # The LAST row-tile is processed as NSPLIT quarter-width ops so the
    # final DVE op rides only a quarter tile behind the last DMA byte.
    NSPLIT = 8
    ND = RT - 1 + NSPLIT      # number of d DMAs == number of DVE ops
    NV = ND
    # rowmin[:, 2i:2i+2] <- the last written word of op i's scan stream;
    # column 2i+1 is the final running-min in both 1x and 2x modes.
    out_ap = nc.dram_tensor("rowmin", [128, 2 * ND], bf16, kind="ExternalOutput").ap()

    proto_tile = nc.alloc_sbuf_tensor("proto_tile", [128, P], bf16).ap()
    labels_tile = nc.alloc_sbuf_tensor("labels_tile", [128, RT], f32).ap()
    # per-tile scan output (ops into the same tile use disjoint col ranges)
    scr = [nc.alloc_sbuf_tensor(f"scr{t}", [128, P], bf16).ap() for t in range(RT)]
    dbig = nc.alloc_sbuf_tensor("dbig", [128, RT * P], bf16).ap()

    H = P // NSPLIT           # split width of the last tile

    # (tile, col_lo, width) in stream order; col offsets are into dbig
    work = [(t, 0, P) for t in range(RT - 1)]
    for s in range(NSPLIT):
        work.append((RT - 1, s * H, H))
    # one d DMA per work item (1 MB per full tile; mid-stream rate is
    # identical to fused 2 MB transfers, and per-item gating is simplest)
    dma_plan = [(t * P + lo, w, i) for i, (t, lo, w) in enumerate(work)]
    # op i is gated on the dma covering it
    dma_of_op = {}
    for di, (lo, w, first_op) in enumerate(dma_plan):
        for oi in range(len(work)):
            t, olo, ow = work[oi]
            a = t * P + olo
            if lo <= a < lo + w:
                dma_of_op[oi] = max(dma_of_op.get(oi, 0), di)

    # One semaphore per DMA (a shared sem with cumulative thresholds can
    # fire early when the 16 SDMA engines skew; a dedicated sem == 16 is
    # exact).
    d_sems = [nc.alloc_semaphore(f"d_sem{i}") for i in range(len(dma_plan))]
    pb_sem = nc.alloc_semaphore("pb_sem")
    lab_sem = nc.alloc_semaphore("lab_sem")
    out_sem = nc.alloc_semaphore("out_sem")
    dve_sem = nc.alloc_semaphore("dve_sem")

    with nc.Block() as block:

        @block.sync
        def _(sync):
            # All DMAs ride the single sync HWDGE ring: a separate
            # scalar-ring DMA is starved ~1:12 behind the d-stream by the
            # per-queue-row round-robin (measured: 17 us for 1 MB).  The
            # host pre-replicates proto to [128, P]; an SBUF->SBUF 0-stride
            # broadcast is rejected by bass and GPSIMD partition_broadcast
            # measures 14.5 us (elem-bound) — both slower.
            sync.dma_start(proto_tile[:], pb_ap[:]).then_inc(pb_sem, 16)
            sync.dma_start(labels_tile[:], lab_ap[:]).then_inc(lab_sem, 16)
            for i, (lo, w, _fo) in enumerate(dma_plan):
                sync.dma_start(
                    dbig[:, lo : lo + w], d_ap[:, lo : lo + w]
                ).then_inc(d_sems[i], 16)
            # Gather each op's final scan word as soon as that op retires;
            # only the last one's completion latency lands in the tail.
            for i, (t, lo, w) in enumerate(work):
                sync.wait_ge(dve_sem, i + 1)
                sync.dma_start(
                    out_ap[:, 2 * i : 2 * i + 2],
                    scr[t][:, lo + w - 2 : lo + w],
                ).then_inc(out_sem, 16)
            sync.wait_ge(out_sem, 16 * ND)
            # Reset all kernel semaphores so re-executing the loaded NEFF
            # behaves identically to the first run.
            all_sems = sorted(
                s.num
                for s in [*d_sems, pb_sem, lab_sem, out_sem, dve_sem]
            )
            lo = prev = all_sems[0]
            for n in all_sems[1:] + [None]:
                if n is not None and n == prev + 1:
                    prev = n
                    continue
                sync.sem_clear(range(lo, prev + 1))
                if n is not None:
                    lo = prev = n

        @block.vector
        def _(vector):
            vector.wait_ge(pb_sem, 16)
            vector.wait_ge(lab_sem, 16)
            for i, (t, lo, w) in enumerate(work):
                vector.wait_ge(d_sems[dma_of_op[i]], 16)
                inst = nc.vector._custom_dve(
                    op,
                    out=scr[t][:, lo : lo + w],
                    in0=dbig[:, t * P + lo : t * P + lo + w],
                    in1=proto_tile[:, lo : lo + w],
                    s0=labels_tile[:, t : t + 1],
                    s1=0.5,
                    imm2=float(BIG),
                )
                if USE_2X:
                    inst.ins.perf_max = 1
                inst.then_inc(dve_sem, 1)

    nc.compile()
    _cache["nc"] = nc
    return nc


def _prep_inputs(min_distances, labels, proto_classes):
    import ml_dtypes

    bf16 = ml_dtypes.bfloat16
    d = np.asarray(min_distances, dtype=np.float32).astype(bf16)
    proto = np.asarray(proto_classes, dtype=np.float32).astype(bf16)
    proto_bc = np.ascontiguousarray(np.broadcast_to(proto[None, :], (128, P)))
    labf = np.asarray(labels, dtype=np.float32)
    in_maps = []
    for c in range(NCORES):
        dsh = np.ascontiguousarray(
            d[c * BSH : (c + 1) * BSH]
            .reshape(RT, 128, P)
            .transpose(1, 0, 2)
            .reshape(128, RT * P)
        )
        lsh = np.ascontiguousarray(
            labf[c * BSH : (c + 1) * BSH].reshape(RT, 128).T
        )
        in_maps.append({"d": dsh, "labels_col": lsh, "proto_bc": proto_bc})
    return in_maps


def _run_device(min_distances, labels, proto_classes, trace=False):
    nc = _get_bass()
    from concourse.bass_utils import run_bass_kernel_spmd

    in_maps = _prep_inputs(min_distances, labels, proto_classes)
    return run_bass_kernel_spmd(
        nc, in_maps, core_ids=list(range(NCORES)), trace=trace
    )


def kernel(min_distances, labels, proto_classes):
    res = _run_device(min_distances, labels, proto_classes).results
    # rowmin[:, 2i+1] = final scan value of op i.  Ops 0..RT-2 are tiles
    # 0..RT-2; ops RT-1.. are quarters of tile RT-1 (combine by min).
    # Row = 1024*c + 128*t + p.  bf16 is exact here: a min of bf16 values.
    stats = np.stack(
        [
            np.asarray(res[c]["rowmin"])[:, 1::2].astype(np.float32)
            for c in range(NCORES)
        ]
    )
    t_last = stats[:, :, RT - 1 :].min(axis=2)
    rowmin = np.concatenate([stats[:, :, : RT - 1], t_last[:, :, None]], axis=2)
    rowmin = rowmin.transpose(0, 2, 1).reshape(B).astype(np.float32)
    # mirror the reference's f32 rounding chain exactly:
    # loss_row = f32(128 - f32(128 - dmin))
    inv = (MAX_DIST - rowmin).astype(np.float32)
    loss_rows = (MAX_DIST - inv).astype(np.float32)
    return np.array(loss_rows.mean(dtype=np.float64), dtype=np.float32)
